# revision 7
# baseline (speedup 1.0000x reference)
"""Causal self-attention (B=4, T=2048, C=1024, H=16, D=64) on 8 TRN2 NeuronCores.

Sharding: tensor-parallel over heads - each core owns 2 of the 16 heads.
Per core:
  qkv^T = W_pack.T @ x^T        (x^T streamed, W stationary; q/k/v each [2D, BT])
  S^T   = k_h^T.T @ q_h^T       (per batch, causal blocks only; the two heads
                                 run concurrently as K=64 row-tiles of the PE)
          + TRI.T @ I           (diagonal blocks: -60000 accumulated into the
                                 masked upper triangle while still in PSUM, so
                                 exp() maps it to 0 - no DVE mask op needed)
  P^T   = exp(S^T/sqrt(D))      (no max-subtraction: logits are O(5))
  yu^T  = [v_h | 1].T @ P^T     (ones column accumulates the softmax denom)
  y^T   = yu^T * (1/denom)      (recip via PE row<->col transposes + DVE)
  out_p = y^T.T @ W_proj_rows   (partial over this core's head-rows, fp16)
Host: out = sum over cores of out_p.

The emission is software-pipelined at instruction granularity: the attention
inner loop is ACT(exp)-bound, so qkv matmuls of batch b+1 and proj matmuls
of batch b (one chunk behind) are interleaved into the attention stream of
batch b to fill PE/DVE slack.  A reserve of proj tiles is held back to the
very end so the tail (last chunk normalize chain) overlaps dense PE work and
the HAM clock gate never re-throttles.
"""

import sys

sys.path.insert(0, "/opt/trn_rl_repo")

import numpy as np
import ml_dtypes

import concourse.bass as bass
import concourse.bacc as bacc
import concourse.mybir as mybir
import concourse.tile as tile
from concourse.bass_utils import run_bass_kernel_spmd

BF16 = mybir.dt.bfloat16
F16 = mybir.dt.float16
F32 = mybir.dt.float32
AF = mybir.ActivationFunctionType

N_CORES = 8
N_HEAD = 16
N_EMBD = 1024
HEAD_DIM = N_EMBD // N_HEAD


class Cfg:
    def __init__(self, B=4, T=2048, C=1024, D=64, CH=512, TG=1024):
        self.B, self.T, self.C, self.D, self.CH, self.TG = B, T, C, D, CH, TG
        self.BT = B * T
        self.n_ct = C // 128          # contraction tiles for qkv
        self.nt = T // 128            # 128-row t-tiles per batch
        self.ncw = T // CH            # tq chunks per batch
        self.r = CH // 128            # t-tiles per chunk
        self.ngb = T // TG            # t-groups per batch (qkv phase)
        self.nchp = TG // CH          # chunks per t-group
        assert C % 128 == 0 and T % CH == 0 and CH % 128 == 0 and T % TG == 0
        assert TG % CH == 0 and D == 64
        assert CH // 128 == 4         # nq=4 assumed by the denom gather


def build(cfg: Cfg) -> bacc.Bacc:
    B, T, C, D, CH, TG = cfg.B, cfg.T, cfg.C, cfg.D, cfg.CH, cfg.TG
    BT, n_ct, nt, ncw, r = cfg.BT, cfg.n_ct, cfg.nt, cfg.ncw, cfg.r
    sm_scale = 1.0 / float(np.sqrt(D))
    nq = CH // 128

    nc = bacc.Bacc("TRN2", target_bir_lowering=False, debug=False,
                   num_devices=N_CORES)

    xT_d = nc.dram_tensor("xT", [C, BT], BF16, kind="ExternalInput")
    wq_d = nc.dram_tensor("wq", [128, n_ct * 128], BF16, kind="ExternalInput")
    wk_d = nc.dram_tensor("wk", [128, n_ct * 128], BF16, kind="ExternalInput")
    wv_d = nc.dram_tensor("wv", [128, n_ct * 128], BF16, kind="ExternalInput")
    wp_d = nc.dram_tensor("wp", [128, C], BF16, kind="ExternalInput")
    trm_d = nc.dram_tensor("trm", [128, 128], BF16, kind="ExternalInput")
    idn_d = nc.dram_tensor("idn", [128, 128], BF16, kind="ExternalInput")
    idf_d = nc.dram_tensor("idf", [128, 128], F32, kind="ExternalInput")
    out_d = nc.dram_tensor("outp", [BT, C], F16, kind="ExternalOutput")

    with tile.TileContext(nc) as tc:
        with (
            tc.tile_pool(name="persist", bufs=1) as persist,
            tc.tile_pool(name="xt", bufs=2 * n_ct) as xt_pool,
            tc.tile_pool(name="pp", bufs=6) as p_pool,
            tc.tile_pool(name="vaug", bufs=2) as vaug_pool,
            tc.tile_pool(name="small", bufs=4) as small_pool,
            tc.tile_pool(name="rep", bufs=4) as rep_pool,
            tc.tile_pool(name="tmp1", bufs=4) as tmp_pool,
            tc.tile_pool(name="ob", bufs=6) as ob_pool,
            tc.tile_pool(name="ps_s", bufs=2, space="PSUM") as ps_s,
            tc.tile_pool(name="ps_aux", bufs=2, space="PSUM") as ps_aux,
            tc.tile_pool(name="ps_yu0", bufs=1, space="PSUM") as ps_yu0,
            tc.tile_pool(name="ps_yu1", bufs=1, space="PSUM") as ps_yu1,
        ):
            # ---- persistent SBUF tensors -------------------------------
            qTs = [persist.tile([128, T], BF16, tag=f"qT{b}", name=f"qT{b}")
                   for b in range(B)]
            kTs = [persist.tile([128, T], BF16, tag=f"kT{b}", name=f"kT{b}")
                   for b in range(B)]
            vTs = [persist.tile([128, T], BF16, tag=f"vT{b}", name=f"vT{b}")
                   for b in range(B)]
            yuTs = [persist.tile([128, T], BF16, tag=f"yuT{b}", name=f"yuT{b}")
                    for b in range(B)]
            wq_sb = persist.tile([128, n_ct * 128], BF16, tag="wq")
            wk_sb = persist.tile([128, n_ct * 128], BF16, tag="wk")
            wv_sb = persist.tile([128, n_ct * 128], BF16, tag="wv")
            wp_sb = persist.tile([128, C], BF16, tag="wp")
            trm_sb = persist.tile([128, 128], BF16, tag="trm")
            idn_sb = persist.tile([128, 128], BF16, tag="idn")
            idf_sb = persist.tile([128, 128], F32, tag="idf")
            nc.sync.dma_start(wq_sb[:], wq_d[:])
            nc.sync.dma_start(wk_sb[:], wk_d[:])
            nc.sync.dma_start(wv_sb[:], wv_d[:])
            nc.sync.dma_start(idn_sb[:], idn_d[:])
            nc.sync.dma_start(trm_sb[:], trm_d[:])
            nc.sync.dma_start(idf_sb[:], idf_d[:])
            nc.sync.dma_start(wp_sb[:], wp_d[:])

            # ---- thunk streams -----------------------------------------
            # Each stream is a list of zero-arg emitters; the scheduler
            # interleaves them so each engine's FIFO gets work in an order
            # that keeps all engines fed.

            def qkv_thunks(b):
                """qkv projections for batch b: per t-group, 8 xT DMAs then
                6 units of (8 accumulating matmuls + 1 PSUM->SBUF cast).
                Returns one thunk list per t-group so the scheduler can
                place group 1 half a batch later than group 0."""
                groups = []
                for gl in range(cfg.ngb):
                    thunks = []
                    g0 = b * T + gl * TG
                    l0 = gl * TG
                    xts = []

                    def dma_group(g0=g0, xts=xts, lo=0, hi=n_ct // 2):
                        for ci in range(lo, hi):
                            xt = xt_pool.tile([128, TG], BF16, tag="xt",
                                              name="xt")
                            nc.sync.dma_start(
                                xt[:], xT_d[128 * ci:128 * (ci + 1),
                                            g0:g0 + TG])
                            xts.append(xt)
                    thunks.append(dma_group)
                    thunks.append(lambda g0=g0, xts=xts:
                                  dma_group(g0, xts, n_ct // 2, n_ct))
                    for wsb, dsts in ((wq_sb, qTs), (wk_sb, kTs),
                                      (wv_sb, vTs)):
                        for ch in range(cfg.nchp):
                            def unit(wsb=wsb, dsts=dsts, ch=ch, l0=l0,
                                     xts=xts):
                                ps = ps_aux.tile([128, CH], F32, tag="aux",
                                                 name="ps")
                                for ci in range(n_ct):
                                    nc.tensor.matmul(
                                        ps[:],
                                        wsb[:, 128 * ci:128 * (ci + 1)],
                                        xts[ci][:, ch * CH:(ch + 1) * CH],
                                        start=(ci == 0),
                                        stop=(ci == n_ct - 1))
                                nc.vector.tensor_copy(
                                    dsts[b][:, l0 + ch * CH:
                                            l0 + (ch + 1) * CH], ps[:])
                            thunks.append(unit)
                    groups.append(thunks)
                return groups

            def attn_thunks(b, proj_sink):
                """Attention for batch b. proj_sink(m) is called when t-tile
                m of yuT[b] is final, enabling the proj of that tile.
                Returns (thunks, fin_thunk): fin_thunk is the last chunk's
                normalize chain, to be spliced into the NEXT batch's stream
                (or the tail) so its PE transposes never head-block."""
                qT, kT, vT, yuT = qTs[b], kTs[b], vTs[b], yuTs[b]
                thunks = []
                va = {}

                def prep(va=va):
                    v = vaug_pool.tile([128, nt * 130], BF16, tag="vaug",
                                       name="va")
                    ones = v.rearrange("p (i h c) -> p i h c",
                                       h=2, c=65)[:, :, :, 64]
                    nc.vector.memset(ones, 1.0)
                    va["t"] = v
                thunks.append(prep)

                def vtrans(i, va=va):
                    # one [128,128] PE transpose covers both heads' v
                    vtp = ps_aux.tile([128, 128], BF16, tag="aux",
                                      name="vtp")
                    nc.tensor.transpose(
                        vtp[:], vT[:, 128 * i:128 * (i + 1)], idn_sb[:])
                    dst = va["t"].rearrange("p (i h c) -> p i h c",
                                            h=2, c=65)[:, i, :, 0:64]
                    src = vtp.rearrange("k (h d) -> k h d", d=64)
                    nc.vector.tensor_copy(dst, src)

                state = {}

                def s_exp(j, i, state=state):
                    c0 = 128 * (i - r * j) if i >= r * j else 0
                    w = CH - c0
                    diag = i >= r * j
                    ss = ps_s.tile([128, 2 * CH], F32, tag="s", name="ss")
                    for h in (0, 1):
                        nc.tensor.matmul(
                            ss[:, h * CH:h * CH + w],
                            kT[64 * h:64 * h + 64, 128 * i:128 * (i + 1)],
                            qT[64 * h:64 * h + 64,
                               CH * j + c0:CH * (j + 1)],
                            start=True, stop=not diag)
                    if diag:
                        # accumulate -60000 into the masked (strictly
                        # future) triangle of the diagonal 128-block so exp
                        # maps it to 0; stays on the PE, no DVE op
                        for h in (0, 1):
                            nc.tensor.matmul(
                                ss[:, h * CH:h * CH + 128],
                                trm_sb[:], idn_sb[:],
                                start=False, stop=True)
                    pt_ = p_pool.tile([128, 2 * w], BF16, tag="p",
                                      name="pt_")
                    if w == CH:
                        nc.scalar.activation(pt_[:], ss[:], AF.Exp,
                                             scale=sm_scale)
                    else:
                        sv = ss.rearrange("p (h c) -> p h c",
                                          c=CH)[:, :, 0:w]
                        pv = pt_.rearrange("p (h c) -> p h c", c=w)
                        nc.scalar.activation(pv, sv, AF.Exp, scale=sm_scale)
                    state[(j, i)] = pt_

                def pv(j, i, i_max, va=va, state=state):
                    c0 = 128 * (i - r * j) if i >= r * j else 0
                    w = CH - c0
                    pt_ = state.pop((j, i))
                    yub = state[("yu", j)]
                    for h in (0, 1):
                        nc.tensor.matmul(
                            yub[h][:, c0:CH],
                            va["t"][:, 130 * i + 65 * h:
                                    130 * i + 65 * h + 65],
                            pt_[:, h * w:(h + 1) * w],
                            start=(i == 0), stop=(i == i_max))

                def final_a(j, state=state):
                    yub = state.pop(("yu", j))
                    # stage yu (+denom row) out of PSUM, then gather both
                    # denom rows into one [8,128] tile for a single transpose
                    yus = []
                    for h in (0, 1):
                        yc = small_pool.tile([65, CH], F32, tag=f"yus{h}",
                                             name=f"yus{h}")
                        nc.vector.tensor_copy(yc[:], yub[h][:])
                        yus.append(yc)
                    dcol = small_pool.tile([8, 128], F32, tag="dcol",
                                           name="dcol")
                    for h in (0, 1):
                        nc.sync.dma_start(dcol[4 * h:4 * h + 4, :],
                                          yus[h][64:65, :])
                    state[("fin", j)] = (dcol, yus)

                def final_b(j, state=state):
                    dcol, yus = state.pop(("fin", j))
                    dt = ps_aux.tile([128, 8], F32, tag="aux", name="dt")
                    nc.tensor.transpose(dt[:], dcol[:], idf_sb[0:8, 0:8])
                    rcol = small_pool.tile([128, 8], F32, tag="rcol")
                    nc.vector.reciprocal(rcol[:], dt[:])
                    rb = ps_aux.tile([8, 128], F32, tag="aux", name="rb")
                    nc.tensor.transpose(rb[:], rcol[:], idf_sb[:, :])
                    rbs = small_pool.tile([8, 128], F32, tag="rbs")
                    nc.vector.tensor_copy(rbs[:], rb[:])
                    rec2 = small_pool.tile([1, 2 * CH], F32, tag="rec2",
                                           name="rec2")
                    nc.sync.dma_start(rec2[0:1, :], rbs[:, :])
                    rep2 = rep_pool.tile([64, 2 * CH], F32, tag="rep",
                                         name="rep2")
                    nc.gpsimd.partition_broadcast(rep2[:], rec2[0:1, :])
                    cols = slice(CH * j, CH * (j + 1))
                    nc.vector.tensor_mul(
                        yuT[0:64, cols], yus[0][0:64, :], rep2[:, 0:CH])
                    tm = tmp_pool.tile([64, CH], BF16, tag="tmp1")
                    nc.vector.tensor_mul(tm[:], yus[1][0:64, :],
                                         rep2[:, CH:2 * CH])
                    nc.sync.dma_start(yuT[64:128, cols], tm[:])

                # stitch the per-chunk streams with PV lagging one i-tile;
                # the denominator/normalize chain of chunk j overlaps the
                # first steps of chunk j+1 so the PE FIFO never head-blocks
                def enable_proj(j):
                    for m in range(nq * j, nq * (j + 1)):
                        proj_sink(m)

                marks = []
                for j in range(ncw):
                    i_max = r * (j + 1) - 1
                    marks.append(len(thunks))

                    def chunk_start(j=j, state=state):
                        state[("yu", j)] = [
                            ps_yu0.tile([65, CH], F32, tag="yu0",
                                        name="yu0"),
                            ps_yu1.tile([65, CH], F32, tag="yu1",
                                        name="yu1")]
                    thunks.append(chunk_start)
                    for k in range(4):
                        thunks.append(lambda i=r * j + k: vtrans(i))
                    for i in range(i_max + 1):
                        def step(j=j, i=i, i_max=i_max):
                            s_exp(j, i)
                            if i > 0:
                                pv(j, i - 1, i_max)
                        # fill weight ~ PE-idle estimate: diagonal steps
                        # have less S/PV streaming, so they take more fill
                        c0 = 128 * (i - r * j) if i >= r * j else 0
                        step.weight = 1.0 + 1.5 * (c0 / CH)
                        thunks.append(step)
                        if j > 0 and i == i_max - 1:
                            # the previous chunk's normalize runs here, a
                            # full chunk after its dcol DMA was dispatched,
                            # so the PE transposes inside never head-block
                            def fin_prev(j=j):
                                final_b(j - 1)
                                enable_proj(j - 1)
                            thunks.append(fin_prev)

                    def tail(j=j, i_max=i_max):
                        pv(j, i_max, i_max)
                        final_a(j)
                    thunks.append(tail)

                def fin_last(j=ncw - 1):
                    final_b(j)
                    enable_proj(j)
                return thunks, fin_last, marks

            # global proj work queue: halves append as their yuT chunk
            # normalizes; the scheduler drains a couple per attention thunk
            proj_pending = []

            proj_obs = {}

            def proj_emit_half(b, m, ch):
                if ch == 0:
                    ob = ob_pool.tile([128, C], F16, tag="ob", name="ob")
                    proj_obs[(b, m)] = ob
                else:
                    ob = proj_obs.pop((b, m))
                po = ps_aux.tile([128, CH], F32, tag="aux", name="po")
                nc.tensor.matmul(
                    po[:],
                    yuTs[b][:, 128 * m:128 * (m + 1)],
                    wp_sb[:, ch * CH:(ch + 1) * CH],
                    start=True, stop=True)
                if (2 * m + ch) % 4 == 3:
                    nc.scalar.copy(ob[:, ch * CH:(ch + 1) * CH], po[:])
                else:
                    nc.vector.tensor_copy(ob[:, ch * CH:(ch + 1) * CH],
                                          po[:])
                if ch == 1:
                    nc.sync.dma_start(
                        out_d[b * T + 128 * m:b * T + 128 * (m + 1), :],
                        ob[:])

            drain_clock = [0]

            def proj_sink_for(b):
                def sink(m):
                    # halves become drain-eligible only ~8 attention thunks
                    # after their normalize chain was issued, so they never
                    # head-block the PE FIFO while the chain is in flight
                    proj_pending.append((drain_clock[0] + 8, b, m, 0))
                    proj_pending.append((drain_clock[0] + 8, b, m, 1))
                return sink

            # ---- interleaved scheduler ---------------------------------
            # warmup: ~7us of throwaway matmuls on a scratch tile (no DMA
            # dependency, so HAM unthrottles while weights are in flight)
            wsc = tmp_pool.tile([128, CH], BF16, tag="wsc", name="wsc")
            nc.vector.memset(wsc[:], 0.5)
            warm = ps_aux.tile([128, CH], F32, tag="aux", name="warm")
            for _ in range(24):
                nc.tensor.matmul(warm[:], wsc[:, 0:128], wsc[:],
                                 start=True, stop=True)

            attn_streams = []
            fin_lasts = []
            chunk_marks = []
            for b in range(B):
                th, fl, marks = attn_thunks(b, proj_sink_for(b))
                attn_streams.append(th)
                fin_lasts.append(fl)
                chunk_marks.append(marks)
            qkv_groups = [qkv_thunks(b) for b in range(B)]
            proj_cursor = [0]

            RESERVE = 24   # proj halves held back for the tail

            def drain_proj(limit, force=False, reserve=0):
                n = 0
                while proj_cursor[0] < len(proj_pending) - reserve \
                        and n < limit:
                    at, b, m, ch = proj_pending[proj_cursor[0]]
                    if not force and at > drain_clock[0]:
                        break
                    proj_emit_half(b, m, ch)
                    proj_cursor[0] += 1
                    n += 1

            # prologue: batch 0's group-0 qkv runs alone before attention
            for th in qkv_groups[0][0]:
                th()
            # Each batch window carries two paced fill zones:
            #   zone A (chunks 0-1): this batch's OWN group-1 qkv (it must
            #     finish before chunk 2 reads q/k/v cols >= TG)
            #   zone B (chunks 2-3): the NEXT batch's group-0 qkv
            # This shifts qkv half a batch later than the naive schedule so
            # batch B-1's ACT-bound attention still has PE fill work.
            for b in range(B):
                attn = list(attn_streams[b])
                zone_split = chunk_marks[b][2]
                if b > 0:
                    # splice the previous batch's last-chunk normalize in
                    # after a few steps (dcol DMA has landed by then)
                    attn.insert(8, fin_lasts[b - 1])
                    if zone_split >= 8:
                        zone_split += 1
                fill_a = list(qkv_groups[b][1])
                fill_b = list(qkv_groups[b + 1][0]) if b + 1 < B else []
                zones = [(0, zone_split, fill_a),
                         (zone_split, len(attn), fill_b)]
                for z0, z1, fill in zones:
                    sub = attn[z0:z1]
                    wsum = sum(getattr(th, "weight", 1.0) for th in sub)
                    rate = len(fill) / wsum if wsum else 0.0
                    credit = 0.0
                    qpos = 0
                    for th in sub:
                        th()
                        drain_clock[0] += 1
                        credit += rate * getattr(th, "weight", 1.0)
                        while credit >= 1.0 and qpos < len(fill):
                            fill[qpos]()
                            qpos += 1
                            credit -= 1.0
                        drain_proj(limit=3, reserve=RESERVE)
                    while qpos < len(fill):
                        fill[qpos]()
                        qpos += 1
            # ---- tail: drain part of the reserve (dense, dependency-free
            # PE work) while the last chunk's dcol DMA lands, then run its
            # normalize and flush everything
            drain_proj(limit=10, force=True)
            fin_lasts[B - 1]()
            drain_proj(limit=10 ** 9, force=True)

    nc.compile()
    return nc


def host_inputs(cfg: Cfg, x, W_attn, W_proj, h0, hpc=2):
    """Per-core input dict for the core owning heads [h0, h0+hpc)."""
    C, D = cfg.C, cfg.D
    assert hpc * D == 128
    bf = ml_dtypes.bfloat16

    def wpack(Wcols):  # [C, 128] -> [128, n_ct*128] (c-tile minor)
        return np.ascontiguousarray(
            Wcols.reshape(cfg.n_ct, 128, 128).transpose(1, 0, 2)
            .reshape(128, cfg.n_ct * 128)).astype(bf)

    cols = np.concatenate([np.arange(h * D, (h + 1) * D)
                           for h in range(h0, h0 + hpc)])
    trm = np.triu(np.ones((128, 128)), k=1) * -60000.0
    return {
        "wq": wpack(W_attn[:, cols]),
        "wk": wpack(W_attn[:, C + cols]),
        "wv": wpack(W_attn[:, 2 * C + cols]),
        "wp": np.ascontiguousarray(W_proj[cols, :]).astype(bf),
        "trm": trm.astype(bf),
        "idn": np.eye(128).astype(bf),
        "idf": np.eye(128, dtype=np.float32),
    }


_NC_CACHE = {}


def kernel(x, W_attn, W_proj):
    x = np.asarray(x)
    W_attn = np.asarray(W_attn)
    W_proj = np.asarray(W_proj)
    B, T, C = x.shape
    cfg = Cfg(B=B, T=T, C=C)
    key = (B, T, C)
    if key not in _NC_CACHE:
        _NC_CACHE[key] = build(cfg)
    nc = _NC_CACHE[key]

    xT = np.ascontiguousarray(x.reshape(cfg.BT, C).T).astype(ml_dtypes.bfloat16)
    in_maps = []
    for core in range(N_CORES):
        m = host_inputs(cfg, x, W_attn, W_proj, h0=2 * core)
        m["xT"] = xT
        in_maps.append(m)

    res = run_bass_kernel_spmd(nc, in_maps, core_ids=list(range(N_CORES)))
    out = np.zeros((cfg.BT, C), dtype=np.float64)
    for core in range(N_CORES):
        out += res.results[core]["outp"].astype(np.float64)
    return out.astype(np.float32).reshape(B, T, C)
